# revision 6
# baseline (speedup 1.0000x reference)
"""Trainium2 Bass kernel for CausalDownsamplingLRU.

Algorithm (per core = one batch element; 8 cores, data-parallel over batch):
  1. Input GEMMs (fp16, PE):  Bu^T[n,t] = (gamma*B)^T.T @ x^T   (re & im)
  2. Phase twist (DVE/GPSIMD): e = exp(-i*j*theta) (.) Bu   -- with lam = r*e^{i theta},
     the twisted recurrence  h~_t = r*h~_{t-1} + e_t  has a REAL per-channel
     coefficient, so re/im decouple into two tensor_tensor_scan ops.
     Twist is chunk-local (j = t mod L); scan carries between chunks are
     rotated by e^{i*L*theta}.
  3. Scans (fp32 state): tensor_tensor_scan along free dim per n-partition.
  4. Untwist + output GEMMs for the last DS steps only:
     y^T = C_re^T.T @ h_re + (-C_im^T).T @ h_im + D^T.T @ x^T
"""
import numpy as np

import concourse.bass as bass
import concourse.mybir as mybir
from concourse.tile import TileContext
from concourse.bass_utils import run_bass_kernel_spmd

BATCH, T, IN, OUT, N = 8, 2048, 512, 512, 512
DS = 1024
P = 128
NB = N // P    # 4 state blocks
IBN = IN // P  # 4 input blocks
OBN = OUT // P # 4 output blocks
L = 1024       # twist/scan chunk length
NCH = T // L   # 2 chunks
HH = 512       # matmul moving free-dim (PSUM bank limit)
NH = L // HH   # 2 sub-chunks per chunk

f32 = mybir.dt.float32
f16 = mybir.dt.float16
AOP = mybir.AluOpType

# which ops run on gpsimd (vs vector): tune for engine balance
GP_TT = False    # twist/untwist partner products
GP_SCAN = False  # imaginary-plane scans

_CACHE = {}


def _build_nc():
    if "nc" in _CACHE:
        return _CACHE["nc"]
    nc = bass.Bass()
    xT = nc.dram_tensor("xT", [IN, T], f16, kind="ExternalInput")
    btr = nc.dram_tensor("btr", [IN, N], f16, kind="ExternalInput")
    bti = nc.dram_tensor("bti", [IN, N], f16, kind="ExternalInput")
    ctr = nc.dram_tensor("ctr", [N, OUT], f16, kind="ExternalInput")
    ctin = nc.dram_tensor("ctin", [N, OUT], f16, kind="ExternalInput")
    dtw = nc.dram_tensor("dtw", [IN, OUT], f16, kind="ExternalInput")
    cosj = nc.dram_tensor("cosj", [N, L], f16, kind="ExternalInput")
    sinj = nc.dram_tensor("sinj", [N, L], f16, kind="ExternalInput")
    rb = nc.dram_tensor("rb", [N, L], f32, kind="ExternalInput")
    # rot columns: 0=cos(L*theta), 1=-sin(L*theta), 2=sin(L*theta)
    rot = nc.dram_tensor("rot", [N, 3], f32, kind="ExternalInput")
    yT = nc.dram_tensor("yT", [OUT, DS], f32, kind="ExternalOutput")

    with TileContext(nc) as tc:
        with (
            tc.tile_pool(name="const", bufs=1) as cp,
            tc.tile_pool(name="xt", bufs=2 * IBN) as xp,
            tc.tile_pool(name="bups", bufs=4, space="PSUM") as bp,
            tc.tile_pool(name="bu", bufs=2) as up,
            tc.tile_pool(name="tw", bufs=2) as wp,
            tc.tile_pool(name="hh", bufs=1) as hp,
            tc.tile_pool(name="carry", bufs=1) as kp,
            tc.tile_pool(name="yps", bufs=2, space="PSUM") as yp,
            tc.tile_pool(name="ysb", bufs=2) as op_,
        ):
            # ---- constants ----
            def load_const(dram, rows, dtype, tagp):
                tiles = []
                for i in range(rows // P):
                    t = cp.tile([P, dram.shape[1]], dtype, tag=f"{tagp}{i}", name=f"{tagp}{i}")
                    nc.sync.dma_start(t[:], dram[i * P:(i + 1) * P, :])
                    tiles.append(t)
                return tiles

            btr_t = load_const(btr, IN, f16, "btr")
            bti_t = load_const(bti, IN, f16, "bti")
            ctr_t = load_const(ctr, N, f16, "ctr")
            ctin_t = load_const(ctin, N, f16, "ctin")
            dtw_t = load_const(dtw, IN, f16, "dtw")
            cos_t = load_const(cosj, N, f16, "cos")
            sin_t = load_const(sinj, N, f16, "sin")
            rb_t = load_const(rb, N, f32, "rb")
            rot_t = load_const(rot, N, f32, "rot")

            c_re = [kp.tile([P, 1], f32, tag=f"cre{nb}", name=f"cre{nb}") for nb in range(NB)]
            c_im = [kp.tile([P, 1], f32, tag=f"cim{nb}", name=f"cim{nb}") for nb in range(NB)]
            t_rr = [kp.tile([P, 1], f32, tag=f"trr{nb}", name=f"trr{nb}") for nb in range(NB)]
            t_ir = [kp.tile([P, 1], f32, tag=f"tir{nb}", name=f"tir{nb}") for nb in range(NB)]

            for c in range(NCH):
                # ---- load x^T chunk ----
                xts = []
                for ib in range(IBN):
                    xt_t = xp.tile([P, L], f16, tag="xt", name="xt")
                    nc.sync.dma_start(xt_t[:], xT[ib * P:(ib + 1) * P, c * L:(c + 1) * L])
                    xts.append(xt_t)

                hh_re, hh_im = [], []
                for nb in range(NB):
                    nsl = slice(nb * P, (nb + 1) * P)
                    # ---- input GEMMs -> Bu (fp16 in SBUF) ----
                    bu = {}
                    for pl, bt_tiles in (("r", btr_t), ("i", bti_t)):
                        bu_t = up.tile([P, L], f16, tag=f"bu{pl}", name=f"bu{pl}")
                        for h in range(NH):
                            ps = bp.tile([P, HH], f32, tag="bups")
                            for ib in range(IBN):
                                nc.tensor.matmul(
                                    ps[:],
                                    bt_tiles[ib][:, nsl],
                                    xts[ib][:, h * HH:(h + 1) * HH],
                                    start=(ib == 0),
                                    stop=(ib == IBN - 1),
                                )
                            nc.scalar.copy(bu_t[:, h * HH:(h + 1) * HH], ps[:])
                        bu[pl] = bu_t

                    # ---- twist: e = exp(-i j theta) * Bu ----
                    p1 = wp.tile([P, L], f16, tag="p1")
                    p2 = wp.tile([P, L], f16, tag="p2")
                    p3 = wp.tile([P, L], f16, tag="p3")
                    p4 = wp.tile([P, L], f16, tag="p4")
                    e_re = wp.tile([P, L], f16, tag="ere")
                    e_im = wp.tile([P, L], f16, tag="eim")
                    eng2 = nc.gpsimd if GP_TT else nc.vector
                    nc.vector.tensor_tensor(p1[:], cos_t[nb][:], bu["r"][:], AOP.mult)
                    eng2.tensor_tensor(p2[:], sin_t[nb][:], bu["i"][:], AOP.mult)
                    nc.vector.tensor_tensor(e_re[:], p1[:], p2[:], AOP.add)
                    nc.vector.tensor_tensor(p3[:], cos_t[nb][:], bu["i"][:], AOP.mult)
                    eng2.tensor_tensor(p4[:], sin_t[nb][:], bu["r"][:], AOP.mult)
                    nc.vector.tensor_tensor(e_im[:], p3[:], p4[:], AOP.subtract)

                    # ---- scans (real coefficient r per partition) ----
                    h_re = wp.tile([P, L], f16, tag="hre")
                    h_im = wp.tile([P, L], f16, tag="him")
                    init_re = 0.0 if c == 0 else c_re[nb][:, 0:1]
                    init_im = 0.0 if c == 0 else c_im[nb][:, 0:1]
                    nc.vector.tensor_tensor_scan(
                        h_re[:], rb_t[nb][:], e_re[:], init_re, AOP.mult, AOP.add)
                    seng = nc.gpsimd if GP_SCAN else nc.vector
                    seng.tensor_tensor_scan(
                        h_im[:], rb_t[nb][:], e_im[:], init_im, AOP.mult, AOP.add)

                    if c < NCH - 1:
                        # ---- carry rotation by e^{i L theta} ----
                        last = slice(L - 1, L)
                        nc.scalar.mul(t_rr[nb][:], h_re[:, last], rot_t[nb][:, 0:1])
                        nc.scalar.mul(c_re[nb][:], h_im[:, last], rot_t[nb][:, 1:2])
                        nc.vector.tensor_tensor(c_re[nb][:], c_re[nb][:], t_rr[nb][:], AOP.add)
                        nc.scalar.mul(t_ir[nb][:], h_im[:, last], rot_t[nb][:, 0:1])
                        nc.scalar.mul(c_im[nb][:], h_re[:, last], rot_t[nb][:, 2:3])
                        nc.vector.tensor_tensor(c_im[nb][:], c_im[nb][:], t_ir[nb][:], AOP.add)

                    if c >= NCH - DS // L:
                        # ---- untwist: hh = exp(+i j theta) * h ----
                        q1 = wp.tile([P, L], f16, tag="q1")
                        q2 = wp.tile([P, L], f16, tag="q2")
                        q3 = wp.tile([P, L], f16, tag="q3")
                        q4 = wp.tile([P, L], f16, tag="q4")
                        hhr = hp.tile([P, L], f16, tag=f"hhr{nb}", name=f"hhr{nb}")
                        hhi = hp.tile([P, L], f16, tag=f"hhi{nb}", name=f"hhi{nb}")
                        nc.vector.tensor_tensor(q1[:], cos_t[nb][:], h_re[:], AOP.mult)
                        eng2.tensor_tensor(q2[:], sin_t[nb][:], h_im[:], AOP.mult)
                        nc.vector.tensor_tensor(hhr[:], q1[:], q2[:], AOP.subtract)
                        nc.vector.tensor_tensor(q3[:], cos_t[nb][:], h_im[:], AOP.mult)
                        eng2.tensor_tensor(q4[:], sin_t[nb][:], h_re[:], AOP.mult)
                        nc.vector.tensor_tensor(hhi[:], q3[:], q4[:], AOP.add)
                        hh_re.append(hhr)
                        hh_im.append(hhi)

                if c >= NCH - DS // L:
                    # ---- output GEMMs ----
                    toff = (c - (NCH - DS // L)) * L
                    for ob in range(OBN):
                        osl = slice(ob * P, (ob + 1) * P)
                        for h in range(NH):
                            hsl = slice(h * HH, (h + 1) * HH)
                            ps = yp.tile([P, HH], f32, tag="yps")
                            nmm = 2 * NB + IBN
                            k = 0
                            for nb in range(NB):
                                nc.tensor.matmul(ps[:], ctr_t[nb][:, osl], hh_re[nb][:, hsl],
                                                 start=(k == 0), stop=(k == nmm - 1))
                                k += 1
                            for nb in range(NB):
                                nc.tensor.matmul(ps[:], ctin_t[nb][:, osl], hh_im[nb][:, hsl],
                                                 start=False, stop=(k == nmm - 1))
                                k += 1
                            for ib in range(IBN):
                                nc.tensor.matmul(ps[:], dtw_t[ib][:, osl], xts[ib][:, hsl],
                                                 start=False, stop=(k == nmm - 1))
                                k += 1
                            ysb = op_.tile([P, HH], f32, tag="ysb")
                            nc.scalar.copy(ysb[:], ps[:])
                            nc.sync.dma_start(
                                yT[osl, toff + h * HH:toff + (h + 1) * HH], ysb[:])

    _legalize_waits(nc)
    nc.finalize()
    _CACHE["nc"] = nc
    return nc


def _legalize_waits(nc):
    """This toolchain's walrus accepts only ONE sync-wait per instruction
    (NEURON_ISA_TPB_EVENTS has a single wait slot); Tile's scheduler can emit
    several. Splice wait-carrying NoOps immediately before each offender —
    semantically identical blocking point, one wait per instruction."""
    cnt = 0
    for f in nc.m.functions:
        for bb in f.blocks:
            out = []
            changed = False
            for ins in bb.instructions:
                si = ins.sync_info
                waits = list(si.on_wait) if si and si.on_wait else []
                if len(waits) > 1:
                    changed = True
                    for w in waits[:-1]:
                        nop = mybir.InstNoOp(name=f"waitnop-{cnt}")
                        cnt += 1
                        nop.engine = ins.engine
                        nop.sync_info = mybir.SyncInfo(on_wait=[w], on_update=[])
                        nc.register_instruction(nop)
                        out.append(nop)
                    ins.sync_info = mybir.SyncInfo(
                        on_wait=[waits[-1]], on_update=list(si.on_update or []))
                out.append(ins)
            if changed:
                bb.instructions = out


def _host_prep(x, nu_log, theta_log, gamma_log, B_re, B_im, C_re, C_im, D):
    f64 = np.float64
    nu = np.asarray(nu_log, f64)
    th = np.asarray(theta_log, f64)
    gl = np.asarray(gamma_log, f64)
    r = np.exp(-np.exp(nu))
    theta = np.exp(th)
    gamma = np.exp(gl)

    shared = {
        "btr": np.ascontiguousarray((gamma[:, None] * np.asarray(B_re, f64)).T).astype(np.float16),
        "bti": np.ascontiguousarray((gamma[:, None] * np.asarray(B_im, f64)).T).astype(np.float16),
        "ctr": np.ascontiguousarray(np.asarray(C_re, f64).T).astype(np.float16),
        "ctin": np.ascontiguousarray((-np.asarray(C_im, f64)).T).astype(np.float16),
        "dtw": np.ascontiguousarray(np.asarray(D, f64).T).astype(np.float16),
    }
    j = np.arange(L, dtype=f64)
    ang = theta[:, None] * j[None, :]
    shared["cosj"] = np.cos(ang).astype(np.float16)
    shared["sinj"] = np.sin(ang).astype(np.float16)
    shared["rb"] = np.ascontiguousarray(
        np.broadcast_to(r[:, None].astype(np.float32), (N, L)))
    shared["rot"] = np.stack(
        [np.cos(L * theta), -np.sin(L * theta), np.sin(L * theta)], axis=1
    ).astype(np.float32)

    x = np.asarray(x, np.float32)
    in_maps = []
    for b in range(BATCH):
        m = dict(shared)
        m["xT"] = np.ascontiguousarray(x[b].T).astype(np.float16)
        in_maps.append(m)
    return in_maps


def _run(in_maps, trace=False):
    nc = _build_nc()
    return run_bass_kernel_spmd(nc, in_maps, core_ids=list(range(BATCH)), trace=trace)


def kernel(**inputs):
    in_maps = _host_prep(**inputs)
    res = _run(in_maps, trace=False)
    y = np.stack([np.ascontiguousarray(res.results[b]["yT"].T) for b in range(BATCH)])
    return y.astype(np.float32)


def kernel_traced(**inputs):
    """Like kernel() but returns (y, exec_time_ns). Used by test.py."""
    in_maps = _host_prep(**inputs)
    res = _run(in_maps, trace=True)
    y = np.stack([np.ascontiguousarray(res.results[b]["yT"].T) for b in range(BATCH)])
    return y.astype(np.float32), res.exec_time_ns


# revision 9
# speedup vs baseline: 1.0026x; 1.0026x over previous
"""Trainium2 Bass kernel for CausalDownsamplingLRU.

Algorithm (per core = one batch element; 8 cores, data-parallel over batch):
  1. Input GEMMs (fp16, PE):  Bu^T[n,t] = (gamma*B)^T.T @ x^T   (re & im)
  2. Phase twist (DVE/GPSIMD): e = exp(-i*j*theta) (.) Bu   -- with lam = r*e^{i theta},
     the twisted recurrence  h~_t = r*h~_{t-1} + e_t  has a REAL per-channel
     coefficient, so re/im decouple into two tensor_tensor_scan ops.
     Twist is chunk-local (j = t mod L); scan carries between chunks are
     rotated by e^{i*L*theta}.
  3. Scans (fp32 state): tensor_tensor_scan along free dim per n-partition.
  4. Untwist + output GEMMs for the last DS steps only:
     y^T = C_re^T.T @ h_re + (-C_im^T).T @ h_im + D^T.T @ x^T
"""
import numpy as np

import concourse.bass as bass
import concourse.mybir as mybir
from concourse.tile import TileContext
from concourse.bass_utils import run_bass_kernel_spmd

BATCH, T, IN, OUT, N = 8, 2048, 512, 512, 512
DS = 1024
P = 128
NB = N // P    # 4 state blocks
IBN = IN // P  # 4 input blocks
OBN = OUT // P # 4 output blocks
L = 1024       # twist/scan chunk length
NCH = T // L   # 2 chunks
HH = 512       # matmul moving free-dim (PSUM bank limit)
NH = L // HH   # 2 sub-chunks per chunk

f32 = mybir.dt.float32
f16 = mybir.dt.float16
AOP = mybir.AluOpType

# which ops run on gpsimd (vs vector): tune for engine balance
GP_TT = False    # twist/untwist partner products
GP_SCAN = False  # imaginary-plane scans

_CACHE = {}


def _build_nc():
    if "nc" in _CACHE:
        return _CACHE["nc"]
    nc = bass.Bass()
    xT = nc.dram_tensor("xT", [IN, T], f16, kind="ExternalInput")
    btr = nc.dram_tensor("btr", [IN, N], f16, kind="ExternalInput")
    bti = nc.dram_tensor("bti", [IN, N], f16, kind="ExternalInput")
    ctr = nc.dram_tensor("ctr", [N, OUT], f16, kind="ExternalInput")
    ctin = nc.dram_tensor("ctin", [N, OUT], f16, kind="ExternalInput")
    dtw = nc.dram_tensor("dtw", [IN, OUT], f16, kind="ExternalInput")
    cosj = nc.dram_tensor("cosj", [N, L], f16, kind="ExternalInput")
    sinj = nc.dram_tensor("sinj", [N, L], f16, kind="ExternalInput")
    rb = nc.dram_tensor("rb", [N, L], f32, kind="ExternalInput")
    # rot columns: 0=cos(L*theta), 1=-sin(L*theta), 2=sin(L*theta)
    rot = nc.dram_tensor("rot", [N, 3], f32, kind="ExternalInput")
    yT = nc.dram_tensor("yT", [OUT, DS], f32, kind="ExternalOutput")

    with TileContext(nc) as tc:
        with (
            tc.tile_pool(name="const", bufs=1) as cp,
            tc.tile_pool(name="xt", bufs=2 * IBN) as xp,
            tc.tile_pool(name="bups", bufs=4, space="PSUM") as bp,
            tc.tile_pool(name="bu", bufs=2) as up,
            tc.tile_pool(name="tw", bufs=2) as wp,
            tc.tile_pool(name="hh", bufs=1) as hp,
            tc.tile_pool(name="carry", bufs=1) as kp,
            tc.tile_pool(name="yps", bufs=2, space="PSUM") as yp,
            tc.tile_pool(name="ysb", bufs=2) as op_,
        ):
            # ---- constants ----
            def load_const(dram, rows, dtype, tagp):
                tiles = []
                for i in range(rows // P):
                    t = cp.tile([P, dram.shape[1]], dtype, tag=f"{tagp}{i}", name=f"{tagp}{i}")
                    nc.sync.dma_start(t[:], dram[i * P:(i + 1) * P, :])
                    tiles.append(t)
                return tiles

            btr_t = load_const(btr, IN, f16, "btr")
            bti_t = load_const(bti, IN, f16, "bti")
            ctr_t = load_const(ctr, N, f16, "ctr")
            ctin_t = load_const(ctin, N, f16, "ctin")
            dtw_t = load_const(dtw, IN, f16, "dtw")
            cos_t = load_const(cosj, N, f16, "cos")
            sin_t = load_const(sinj, N, f16, "sin")
            rb_t = load_const(rb, N, f32, "rb")
            rot_t = load_const(rot, N, f32, "rot")

            c_re = [kp.tile([P, 1], f32, tag=f"cre{nb}", name=f"cre{nb}") for nb in range(NB)]
            c_im = [kp.tile([P, 1], f32, tag=f"cim{nb}", name=f"cim{nb}") for nb in range(NB)]
            t_rr = [kp.tile([P, 1], f32, tag=f"trr{nb}", name=f"trr{nb}") for nb in range(NB)]
            t_ir = [kp.tile([P, 1], f32, tag=f"tir{nb}", name=f"tir{nb}") for nb in range(NB)]

            for c in range(NCH):
                # ---- load x^T chunk ----
                xts = []
                for ib in range(IBN):
                    xt_t = xp.tile([P, L], f16, tag="xt", name="xt")
                    nc.sync.dma_start(xt_t[:], xT[ib * P:(ib + 1) * P, c * L:(c + 1) * L])
                    xts.append(xt_t)

                hh_re, hh_im = [], []
                for nb in range(NB):
                    nsl = slice(nb * P, (nb + 1) * P)
                    # ---- input GEMMs -> Bu (fp16 in SBUF) ----
                    bu = {}
                    for pl, bt_tiles in (("r", btr_t), ("i", bti_t)):
                        bu_t = up.tile([P, L], f16, tag=f"bu{pl}", name=f"bu{pl}")
                        for h in range(NH):
                            ps = bp.tile([P, HH], f32, tag="bups")
                            for ib in range(IBN):
                                nc.tensor.matmul(
                                    ps[:],
                                    bt_tiles[ib][:, nsl],
                                    xts[ib][:, h * HH:(h + 1) * HH],
                                    start=(ib == 0),
                                    stop=(ib == IBN - 1),
                                )
                            nc.scalar.copy(bu_t[:, h * HH:(h + 1) * HH], ps[:])
                        bu[pl] = bu_t

                    # ---- twist: e = exp(-i j theta) * Bu ----
                    p1 = wp.tile([P, L], f16, tag="p1")
                    p2 = wp.tile([P, L], f16, tag="p2")
                    p3 = wp.tile([P, L], f16, tag="p3")
                    p4 = wp.tile([P, L], f16, tag="p4")
                    e_re = wp.tile([P, L], f16, tag="ere")
                    e_im = wp.tile([P, L], f16, tag="eim")
                    eng2 = nc.gpsimd if GP_TT else nc.vector
                    nc.vector.tensor_tensor(p1[:], cos_t[nb][:], bu["r"][:], AOP.mult)
                    eng2.tensor_tensor(p2[:], sin_t[nb][:], bu["i"][:], AOP.mult)
                    nc.vector.tensor_tensor(e_re[:], p1[:], p2[:], AOP.add)
                    nc.vector.tensor_tensor(p3[:], cos_t[nb][:], bu["i"][:], AOP.mult)
                    eng2.tensor_tensor(p4[:], sin_t[nb][:], bu["r"][:], AOP.mult)
                    nc.vector.tensor_tensor(e_im[:], p3[:], p4[:], AOP.subtract)

                    # ---- scans (real coefficient r per partition) ----
                    h_re = wp.tile([P, L], f16, tag="hre")
                    h_im = wp.tile([P, L], f16, tag="him")
                    init_re = 0.0 if c == 0 else c_re[nb][:, 0:1]
                    init_im = 0.0 if c == 0 else c_im[nb][:, 0:1]
                    nc.vector.tensor_tensor_scan(
                        h_re[:], rb_t[nb][:], e_re[:], init_re, AOP.mult, AOP.add)
                    seng = nc.gpsimd if GP_SCAN else nc.vector
                    seng.tensor_tensor_scan(
                        h_im[:], rb_t[nb][:], e_im[:], init_im, AOP.mult, AOP.add)

                    if c < NCH - 1:
                        # ---- carry rotation by e^{i L theta} ----
                        last = slice(L - 1, L)
                        nc.scalar.mul(t_rr[nb][:], h_re[:, last], rot_t[nb][:, 0:1])
                        nc.scalar.mul(c_re[nb][:], h_im[:, last], rot_t[nb][:, 1:2])
                        nc.vector.tensor_tensor(c_re[nb][:], c_re[nb][:], t_rr[nb][:], AOP.add)
                        nc.scalar.mul(t_ir[nb][:], h_im[:, last], rot_t[nb][:, 0:1])
                        nc.scalar.mul(c_im[nb][:], h_re[:, last], rot_t[nb][:, 2:3])
                        nc.vector.tensor_tensor(c_im[nb][:], c_im[nb][:], t_ir[nb][:], AOP.add)

                    if c >= NCH - DS // L:
                        # ---- untwist: hh = exp(+i j theta) * h ----
                        q1 = wp.tile([P, L], f16, tag="q1")
                        q2 = wp.tile([P, L], f16, tag="q2")
                        q3 = wp.tile([P, L], f16, tag="q3")
                        q4 = wp.tile([P, L], f16, tag="q4")
                        hhr = hp.tile([P, L], f16, tag=f"hhr{nb}", name=f"hhr{nb}")
                        hhi = hp.tile([P, L], f16, tag=f"hhi{nb}", name=f"hhi{nb}")
                        nc.vector.tensor_tensor(q1[:], cos_t[nb][:], h_re[:], AOP.mult)
                        eng2.tensor_tensor(q2[:], sin_t[nb][:], h_im[:], AOP.mult)
                        nc.vector.tensor_tensor(hhr[:], q1[:], q2[:], AOP.subtract)
                        nc.vector.tensor_tensor(q3[:], cos_t[nb][:], h_im[:], AOP.mult)
                        eng2.tensor_tensor(q4[:], sin_t[nb][:], h_re[:], AOP.mult)
                        nc.vector.tensor_tensor(hhi[:], q3[:], q4[:], AOP.add)
                        hh_re.append(hhr)
                        hh_im.append(hhi)

                if c >= NCH - DS // L:
                    # ---- output GEMMs ----
                    toff = (c - (NCH - DS // L)) * L
                    for ob in range(OBN):
                        osl = slice(ob * P, (ob + 1) * P)
                        for h in range(NH):
                            hsl = slice(h * HH, (h + 1) * HH)
                            ps = yp.tile([P, HH], f32, tag="yps")
                            nmm = 2 * NB + IBN
                            k = 0
                            for nb in range(NB):
                                nc.tensor.matmul(ps[:], ctr_t[nb][:, osl], hh_re[nb][:, hsl],
                                                 start=(k == 0), stop=(k == nmm - 1))
                                k += 1
                            for nb in range(NB):
                                nc.tensor.matmul(ps[:], ctin_t[nb][:, osl], hh_im[nb][:, hsl],
                                                 start=False, stop=(k == nmm - 1))
                                k += 1
                            for ib in range(IBN):
                                nc.tensor.matmul(ps[:], dtw_t[ib][:, osl], xts[ib][:, hsl],
                                                 start=False, stop=(k == nmm - 1))
                                k += 1
                            ysb = op_.tile([P, HH], f32, tag="ysb")
                            nc.scalar.copy(ysb[:], ps[:])
                            nc.sync.dma_start(
                                yT[osl, toff + h * HH:toff + (h + 1) * HH], ysb[:])

    _legalize_waits(nc)
    nc.finalize()
    _CACHE["nc"] = nc
    return nc


def _legalize_waits(nc):
    """This toolchain's walrus accepts only ONE sync-wait per instruction
    (NEURON_ISA_TPB_EVENTS has a single wait slot); Tile's scheduler can emit
    several. Splice wait-carrying NoOps immediately before each offender —
    semantically identical blocking point, one wait per instruction."""
    cnt = 0
    for f in nc.m.functions:
        for bb in f.blocks:
            out = []
            changed = False
            for ins in bb.instructions:
                si = ins.sync_info
                waits = list(si.on_wait) if si and si.on_wait else []
                if len(waits) > 1:
                    changed = True
                    for w in waits[:-1]:
                        nop = mybir.InstNoOp(name=f"waitnop-{cnt}")
                        cnt += 1
                        nop.engine = ins.engine
                        nop.sync_info = mybir.SyncInfo(on_wait=[w], on_update=[])
                        nc.register_instruction(nop)
                        out.append(nop)
                    ins.sync_info = mybir.SyncInfo(
                        on_wait=[waits[-1]], on_update=list(si.on_update or []))
                out.append(ins)
            if changed:
                bb.instructions = out


def _host_prep(x, nu_log, theta_log, gamma_log, B_re, B_im, C_re, C_im, D):
    f64 = np.float64
    nu = np.asarray(nu_log, f64)
    th = np.asarray(theta_log, f64)
    gl = np.asarray(gamma_log, f64)
    r = np.exp(-np.exp(nu))
    theta = np.exp(th)
    gamma = np.exp(gl)

    shared = {
        "btr": np.ascontiguousarray((gamma[:, None] * np.asarray(B_re, f64)).T).astype(np.float16),
        "bti": np.ascontiguousarray((gamma[:, None] * np.asarray(B_im, f64)).T).astype(np.float16),
        "ctr": np.ascontiguousarray(np.asarray(C_re, f64).T).astype(np.float16),
        "ctin": np.ascontiguousarray((-np.asarray(C_im, f64)).T).astype(np.float16),
        "dtw": np.ascontiguousarray(np.asarray(D, f64).T).astype(np.float16),
    }
    j = np.arange(L, dtype=f64)
    ang = theta[:, None] * j[None, :]
    shared["cosj"] = np.cos(ang).astype(np.float16)
    shared["sinj"] = np.sin(ang).astype(np.float16)
    shared["rb"] = np.ascontiguousarray(
        np.broadcast_to(r[:, None].astype(np.float32), (N, L)))
    shared["rot"] = np.stack(
        [np.cos(L * theta), -np.sin(L * theta), np.sin(L * theta)], axis=1
    ).astype(np.float32)

    x = np.asarray(x, np.float32)
    in_maps = []
    for b in range(BATCH):
        m = dict(shared)
        m["xT"] = np.ascontiguousarray(x[b].T).astype(np.float16)
        in_maps.append(m)
    return in_maps


def _run(in_maps, trace=False):
    nc = _build_nc()
    return run_bass_kernel_spmd(nc, in_maps, core_ids=list(range(BATCH)), trace=trace)


def kernel(**inputs):
    in_maps = _host_prep(**inputs)
    res = _run(in_maps, trace=False)
    y = np.stack([np.ascontiguousarray(res.results[b]["yT"].T) for b in range(BATCH)])
    return y.astype(np.float32)


def kernel_traced(**inputs):
    """Like kernel() but returns (y, exec_time_ns). Used by test.py."""
    in_maps = _host_prep(**inputs)
    res = _run(in_maps, trace=True)
    y = np.stack([np.ascontiguousarray(res.results[b]["yT"].T) for b in range(BATCH)])
    return y.astype(np.float32), res.exec_time_ns


# revision 11
# speedup vs baseline: 1.2836x; 1.2803x over previous
"""Trainium2 Bass kernel for CausalDownsamplingLRU.

Algorithm (per core = one batch element; 8 cores, data-parallel over batch):
  With lam = r*e^{i theta} (per state n), h_t = lam*h_{t-1} + Bu_t, and only
  y[:, -DS:] needed:

  1. Input GEMMs (fp16, PE): Bu^T[n,t] = (gamma*B)^T.T @ x^T  (re & im planes)
  2. FIRST half (t < 1024): only h_{1023} is needed (the carry into the
     output window). Computed as a weighted reduction
        h_1023 = sum_s lam^{1023-s} Bu_s
     via tensor_tensor_reduce (elementwise mult + free-dim accumulate),
     4 TTR ops per n-block with chained accumulator init.
  3. SECOND half: phase twist e_j = e^{-i j theta} (.) Bu_{1024+j} decouples
     the complex recurrence into two REAL per-partition scans
        s_j = r*s_{j-1} + e_j   (tensor_tensor_scan, fp32 state),
     with initial s_{-1} = e^{i theta} * h_1023.
  4. Untwist h = e^{+i j theta} (.) s, then output GEMMs:
        y^T = C_re^T.T @ h_re + (-C_im^T).T @ h_im + D^T.T @ x^T
"""
import numpy as np

import concourse.bass as bass
import concourse.mybir as mybir
from concourse.tile import TileContext
from concourse.bass_utils import run_bass_kernel_spmd

BATCH, T, IN, OUT, N = 8, 2048, 512, 512, 512
DS = 1024
P = 128
NB = N // P    # 4 state blocks
IBN = IN // P  # 4 input blocks
OBN = OUT // P # 4 output blocks
HF = 1024      # half length (= DS)
HH = 512       # matmul moving free-dim (PSUM bank limit for f32 out)

f32 = mybir.dt.float32
f16 = mybir.dt.float16
AOP = mybir.AluOpType

GP_TT = False   # offload untwist partner products to gpsimd

_CACHE = {}


def _build_nc():
    if "nc" in _CACHE:
        return _CACHE["nc"]
    nc = bass.Bass()
    xT = nc.dram_tensor("xT", [IN, T], f16, kind="ExternalInput")
    btr = nc.dram_tensor("btr", [IN, N], f16, kind="ExternalInput")
    bti = nc.dram_tensor("bti", [IN, N], f16, kind="ExternalInput")
    ctr = nc.dram_tensor("ctr", [N, OUT], f16, kind="ExternalInput")
    ctin = nc.dram_tensor("ctin", [N, OUT], f16, kind="ExternalInput")
    dtw = nc.dram_tensor("dtw", [IN, OUT], f16, kind="ExternalInput")
    vre = nc.dram_tensor("vre", [N, HF], f16, kind="ExternalInput")
    vim = nc.dram_tensor("vim", [N, HF], f16, kind="ExternalInput")
    cosj = nc.dram_tensor("cosj", [N, HF], f16, kind="ExternalInput")
    sinj = nc.dram_tensor("sinj", [N, HF], f16, kind="ExternalInput")
    rb = nc.dram_tensor("rb", [N, HF], f32, kind="ExternalInput")
    # rot columns: 0=cos(theta), 1=-sin(theta), 2=sin(theta)
    rot = nc.dram_tensor("rot", [N, 3], f32, kind="ExternalInput")
    yT = nc.dram_tensor("yT", [OUT, DS], f32, kind="ExternalOutput")

    with TileContext(nc) as tc:
        with (
            tc.tile_pool(name="const", bufs=1) as cp,
            tc.tile_pool(name="xt", bufs=4) as xp,
            tc.tile_pool(name="bups", bufs=3, space="PSUM") as bp,
            tc.tile_pool(name="bu", bufs=2) as up,
            tc.tile_pool(name="tw", bufs=2) as wp,
            tc.tile_pool(name="hh", bufs=1) as hp,
            tc.tile_pool(name="carry", bufs=1) as kp,
            tc.tile_pool(name="yps", bufs=2, space="PSUM") as yp,
            tc.tile_pool(name="ysb", bufs=2) as op_,
        ):
            def load_const(dram, rows, dtype, tagp):
                tiles = []
                for i in range(rows // P):
                    t = cp.tile([P, dram.shape[1]], dtype, tag=f"{tagp}{i}",
                                name=f"{tagp}{i}")
                    nc.sync.dma_start(t[:], dram[i * P:(i + 1) * P, :])
                    tiles.append(t)
                return tiles

            # ---- x + input weights first so PE can start ASAP ----
            xts = []
            for ib in range(IBN):
                xt_t = xp.tile([P, T], f16, tag="xt", name="xt")
                nc.sync.dma_start(xt_t[:], xT[ib * P:(ib + 1) * P, :])
                xts.append(xt_t)
            btr_t = load_const(btr, IN, f16, "btr")
            bti_t = load_const(bti, IN, f16, "bti")
            vre_t = load_const(vre, N, f16, "vre")
            vim_t = load_const(vim, N, f16, "vim")
            cos_t = load_const(cosj, N, f16, "cos")
            sin_t = load_const(sinj, N, f16, "sin")
            rb_t = load_const(rb, N, f32, "rb")
            rot_t = load_const(rot, N, f32, "rot")

            def input_gemm(nb, half, pl_tiles, name):
                """Bu^T[n-block, t in half] -> fp16 SBUF tile [P, HF]."""
                nsl = slice(nb * P, (nb + 1) * P)
                ps = bp.tile([P, HF], f32, tag="bups", name="bups")
                for h in range(HF // HH):
                    t0 = half * HF + h * HH
                    for ib in range(IBN):
                        nc.tensor.matmul(
                            ps[:, h * HH:(h + 1) * HH],
                            pl_tiles[ib][:, nsl],
                            xts[ib][:, t0:t0 + HH],
                            start=(ib == 0),
                            stop=(ib == IBN - 1),
                        )
                bu_t = up.tile([P, HF], f16, tag=name, name=name)
                nc.scalar.copy(bu_t[:], ps[:])
                return bu_t

            # ---- first half: GEMMs + weighted-reduce carries ----
            acc_re, acc_im = [], []
            for nb in range(NB):
                buA_r = input_gemm(nb, 0, btr_t, "buAr")
                buA_i = input_gemm(nb, 0, bti_t, "buAi")
                dump = wp.tile([P, HF], f16, tag="dump", name="dump")
                s1 = kp.tile([P, 1], f32, tag=f"s1{nb}", name=f"s1{nb}")
                s2 = kp.tile([P, 1], f32, tag=f"s2{nb}", name=f"s2{nb}")
                s3 = kp.tile([P, 1], f32, tag=f"s3{nb}", name=f"s3{nb}")
                s4 = kp.tile([P, 1], f32, tag=f"s4{nb}", name=f"s4{nb}")
                a_re = kp.tile([P, 1], f32, tag=f"are{nb}", name=f"are{nb}")
                a_im = kp.tile([P, 1], f32, tag=f"aim{nb}", name=f"aim{nb}")
                # h1023 = sum_s lam^{1023-s} Bu_s  (4 product-sums via STT accum)
                nc.vector.scalar_tensor_tensor(
                    dump[:], vre_t[nb][:], 1.0, buA_r[:], AOP.bypass, AOP.mult,
                    accum_out=s1[:])
                nc.vector.scalar_tensor_tensor(
                    dump[:], vim_t[nb][:], 1.0, buA_i[:], AOP.bypass, AOP.mult,
                    accum_out=s2[:])
                nc.vector.scalar_tensor_tensor(
                    dump[:], vre_t[nb][:], 1.0, buA_i[:], AOP.bypass, AOP.mult,
                    accum_out=s3[:])
                nc.vector.scalar_tensor_tensor(
                    dump[:], vim_t[nb][:], 1.0, buA_r[:], AOP.bypass, AOP.mult,
                    accum_out=s4[:])
                nc.vector.tensor_tensor(a_re[:], s1[:], s2[:], AOP.subtract)
                nc.vector.tensor_tensor(a_im[:], s3[:], s4[:], AOP.add)
                acc_re.append(a_re)
                acc_im.append(a_im)

            # ---- second half: GEMMs + twist + scans + untwist ----
            hh_re, hh_im = [], []
            for nb in range(NB):
                buB_r = input_gemm(nb, 1, btr_t, "buBr")
                buB_i = input_gemm(nb, 1, bti_t, "buBi")

                # carry rotation: init = e^{i theta} * h_1023
                i_re = kp.tile([P, 1], f32, tag=f"ire{nb}", name=f"ire{nb}")
                i_im = kp.tile([P, 1], f32, tag=f"iim{nb}", name=f"iim{nb}")
                u_re = kp.tile([P, 1], f32, tag=f"ure{nb}", name=f"ure{nb}")
                u_im = kp.tile([P, 1], f32, tag=f"uim{nb}", name=f"uim{nb}")
                nc.scalar.mul(u_re[:], acc_re[nb][:], rot_t[nb][:, 0:1])
                nc.vector.scalar_tensor_tensor(
                    i_re[:], acc_im[nb][:], rot_t[nb][:, 1:2], u_re[:],
                    AOP.mult, AOP.add)
                nc.scalar.mul(u_im[:], acc_im[nb][:], rot_t[nb][:, 0:1])
                nc.vector.scalar_tensor_tensor(
                    i_im[:], acc_re[nb][:], rot_t[nb][:, 2:3], u_im[:],
                    AOP.mult, AOP.add)

                # twist: e = e^{-i j theta} * Bu
                p1 = wp.tile([P, HF], f16, tag="p1", name="p1")
                p2 = wp.tile([P, HF], f16, tag="p2", name="p2")
                p3 = wp.tile([P, HF], f16, tag="p3", name="p3")
                p4 = wp.tile([P, HF], f16, tag="p4", name="p4")
                e_re = wp.tile([P, HF], f16, tag="ere", name="ere")
                e_im = wp.tile([P, HF], f16, tag="eim", name="eim")
                nc.vector.tensor_tensor(p1[:], cos_t[nb][:], buB_r[:], AOP.mult)
                nc.vector.tensor_tensor(p2[:], sin_t[nb][:], buB_i[:], AOP.mult)
                nc.vector.tensor_tensor(e_re[:], p1[:], p2[:], AOP.add)
                nc.vector.tensor_tensor(p3[:], cos_t[nb][:], buB_i[:], AOP.mult)
                nc.vector.tensor_tensor(p4[:], sin_t[nb][:], buB_r[:], AOP.mult)
                nc.vector.tensor_tensor(e_im[:], p3[:], p4[:], AOP.subtract)

                # real scans (fp32 state)
                h_re = wp.tile([P, HF], f16, tag="hre", name="hre")
                h_im = wp.tile([P, HF], f16, tag="him", name="him")
                nc.vector.tensor_tensor_scan(
                    h_re[:], rb_t[nb][:], e_re[:], i_re[:, 0:1], AOP.mult, AOP.add)
                nc.vector.tensor_tensor_scan(
                    h_im[:], rb_t[nb][:], e_im[:], i_im[:, 0:1], AOP.mult, AOP.add)

                # untwist: hh = e^{+i j theta} * h
                eng2 = nc.gpsimd if GP_TT else nc.vector
                q1 = wp.tile([P, HF], f16, tag="q1", name="q1")
                q2 = wp.tile([P, HF], f16, tag="q2", name="q2")
                q3 = wp.tile([P, HF], f16, tag="q3", name="q3")
                q4 = wp.tile([P, HF], f16, tag="q4", name="q4")
                hhr = hp.tile([P, HF], f16, tag=f"hhr{nb}", name=f"hhr{nb}")
                hhi = hp.tile([P, HF], f16, tag=f"hhi{nb}", name=f"hhi{nb}")
                nc.vector.tensor_tensor(q1[:], cos_t[nb][:], h_re[:], AOP.mult)
                eng2.tensor_tensor(q2[:], sin_t[nb][:], h_im[:], AOP.mult)
                nc.vector.tensor_tensor(hhr[:], q1[:], q2[:], AOP.subtract)
                nc.vector.tensor_tensor(q3[:], cos_t[nb][:], h_im[:], AOP.mult)
                eng2.tensor_tensor(q4[:], sin_t[nb][:], h_re[:], AOP.mult)
                nc.vector.tensor_tensor(hhi[:], q3[:], q4[:], AOP.add)
                hh_re.append(hhr)
                hh_im.append(hhi)

            # ---- output weights (queue-ordered after input-side DMAs) ----
            ctr_t = load_const(ctr, N, f16, "ctr")
            ctin_t = load_const(ctin, N, f16, "ctin")
            dtw_t = load_const(dtw, IN, f16, "dtw")

            # ---- output GEMMs ----
            for ob in range(OBN):
                osl = slice(ob * P, (ob + 1) * P)
                for h in range(DS // HH):
                    hsl = slice(h * HH, (h + 1) * HH)
                    xsl = slice(HF + h * HH, HF + (h + 1) * HH)
                    ps = yp.tile([P, HH], f32, tag="yps", name="yps")
                    nmm = 2 * NB + IBN
                    k = 0
                    for nb in range(NB):
                        nc.tensor.matmul(ps[:], ctr_t[nb][:, osl], hh_re[nb][:, hsl],
                                         start=(k == 0), stop=(k == nmm - 1))
                        k += 1
                    for nb in range(NB):
                        nc.tensor.matmul(ps[:], ctin_t[nb][:, osl], hh_im[nb][:, hsl],
                                         start=False, stop=(k == nmm - 1))
                        k += 1
                    for ib in range(IBN):
                        nc.tensor.matmul(ps[:], dtw_t[ib][:, osl], xts[ib][:, xsl],
                                         start=False, stop=(k == nmm - 1))
                        k += 1
                    ysb = op_.tile([P, HH], f32, tag="ysb", name="ysb")
                    nc.scalar.copy(ysb[:], ps[:])
                    nc.sync.dma_start(yT[osl, hsl], ysb[:])

    _legalize_waits(nc)
    nc.finalize()
    _CACHE["nc"] = nc
    return nc


def _legalize_waits(nc):
    """This toolchain's walrus accepts only ONE sync-wait per instruction
    (NEURON_ISA_TPB_EVENTS has a single wait slot); Tile's scheduler can emit
    several. Splice wait-carrying NoOps immediately before each offender —
    semantically identical blocking point, one wait per instruction."""
    cnt = 0
    for f in nc.m.functions:
        for bb in f.blocks:
            out = []
            changed = False
            for ins in bb.instructions:
                si = ins.sync_info
                waits = list(si.on_wait) if si and si.on_wait else []
                if len(waits) > 1:
                    changed = True
                    for w in waits[:-1]:
                        nop = mybir.InstNoOp(name=f"waitnop-{cnt}")
                        cnt += 1
                        nop.engine = ins.engine
                        nop.sync_info = mybir.SyncInfo(on_wait=[w], on_update=[])
                        nc.register_instruction(nop)
                        out.append(nop)
                    ins.sync_info = mybir.SyncInfo(
                        on_wait=[waits[-1]], on_update=list(si.on_update or []))
                out.append(ins)
            if changed:
                bb.instructions = out


def _host_prep(x, nu_log, theta_log, gamma_log, B_re, B_im, C_re, C_im, D):
    f64 = np.float64
    nu = np.asarray(nu_log, f64)
    th = np.asarray(theta_log, f64)
    gl = np.asarray(gamma_log, f64)
    r = np.exp(-np.exp(nu))
    theta = np.exp(th)
    gamma = np.exp(gl)

    shared = {
        "btr": np.ascontiguousarray((gamma[:, None] * np.asarray(B_re, f64)).T).astype(np.float16),
        "bti": np.ascontiguousarray((gamma[:, None] * np.asarray(B_im, f64)).T).astype(np.float16),
        "ctr": np.ascontiguousarray(np.asarray(C_re, f64).T).astype(np.float16),
        "ctin": np.ascontiguousarray((-np.asarray(C_im, f64)).T).astype(np.float16),
        "dtw": np.ascontiguousarray(np.asarray(D, f64).T).astype(np.float16),
    }
    j = np.arange(HF, dtype=f64)
    ang = theta[:, None] * j[None, :]
    shared["cosj"] = np.cos(ang).astype(np.float16)
    shared["sinj"] = np.sin(ang).astype(np.float16)
    # V = lam^{1023-s} = r^{1023-s} e^{i (1023-s) theta}
    e = (HF - 1) - j
    mag = np.exp(np.log(r)[:, None] * e[None, :])
    angv = theta[:, None] * e[None, :]
    shared["vre"] = (mag * np.cos(angv)).astype(np.float16)
    shared["vim"] = (mag * np.sin(angv)).astype(np.float16)
    shared["rb"] = np.ascontiguousarray(
        np.broadcast_to(r[:, None].astype(np.float32), (N, HF)))
    shared["rot"] = np.stack(
        [np.cos(theta), -np.sin(theta), np.sin(theta)], axis=1).astype(np.float32)

    x = np.asarray(x, np.float32)
    in_maps = []
    for b in range(BATCH):
        m = dict(shared)
        m["xT"] = np.ascontiguousarray(x[b].T).astype(np.float16)
        in_maps.append(m)
    return in_maps


def _run(in_maps, trace=False):
    nc = _build_nc()
    return run_bass_kernel_spmd(nc, in_maps, core_ids=list(range(BATCH)), trace=trace)


def kernel(**inputs):
    in_maps = _host_prep(**inputs)
    res = _run(in_maps, trace=False)
    y = np.stack([np.ascontiguousarray(res.results[b]["yT"].T) for b in range(BATCH)])
    return y.astype(np.float32)


def kernel_traced(**inputs):
    """Like kernel() but returns (y, exec_time_ns). Used by test.py."""
    in_maps = _host_prep(**inputs)
    res = _run(in_maps, trace=True)
    y = np.stack([np.ascontiguousarray(res.results[b]["yT"].T) for b in range(BATCH)])
    return y.astype(np.float32), res.exec_time_ns
